# revision 8
# baseline (speedup 1.0000x reference)
"""Trainium2 Bass kernel for nn_ArgumentClassification (2-layer BiLSTM tagger).

Sharding: data-parallel over batch B=32 across 8 NeuronCores (4 rows each),
LSTM/Linear weights replicated. No collectives.

Per-core pipeline (all compute on device):
  1. mean over 4 transformer layers of hidden_states  -> x [4,256,768]
  2. predicate-relative delta + role mask features    -> x_ext [4,256,770]
     (x is built directly in transposed layout x.T [770, S*B] via PE transposes)
  3. L0 BiLSTM: input projection (batched matmul over all timesteps, biases
     folded in via a ones-row), then the sequential 256-step scan in
     gates-transposed layout [2048, B] with Whh stationary on the PE.
     Forward/backward directions interleaved so the PE never waits on the
     DVE/ACT gate-nonlinearity tail.
  4. L1 BiLSTM: same, input = [h0f; h0b].
  5. out = h1 @ W_out.T + b_out, PE-transposed back to [B,S,30] and DMA'd out.

Gate order is host-permuted from PyTorch's (i,f,g,o) to (i,f,o,g) so the scan
needs only two activation instructions per step: sigmoid over tiles 0:12 and
tanh over tiles 12:16.
"""
import sys

sys.path.insert(0, "/opt/trn_rl_repo")

import numpy as np
import ml_dtypes

import concourse.bass as bass
import concourse.tile as tile
from concourse import bacc, mybir
from concourse.bass import ds
from concourse.masks import make_identity

BF16 = mybir.dt.bfloat16
F32 = mybir.dt.float32
AF = mybir.ActivationFunctionType
OP = mybir.AluOpType

B, S, E, H, C = 32, 256, 768, 512, 30
NCORES = 8
BL = B // NCORES          # 4 rows per core
SB = S * BL               # 1024 columns, ordered (t, b): col = t*BL + b
G = 4 * H                 # 2048 gate rows
MT = G // 128             # 16 gate m-tiles
KH = H // 128             # 4 hidden k-tiles
K0 = 7                    # L0 input k-tiles ([770 + ones-row] padded to 896)
K1 = 9                    # L1 input k-tiles (1024 + ones-row -> 1152)
UNROLL = 8

_cache = {}


def _bf(a):
    return np.asarray(a, dtype=ml_dtypes.bfloat16)


def _prep_weights(inp):
    """Host-side: permute gates to (i,f,o,g), transpose, pad, fold biases,
    tile for SBUF. Returns dict of name -> np array matching dram params."""
    perm = np.concatenate([
        np.arange(0, H),          # i
        np.arange(H, 2 * H),      # f
        np.arange(3 * H, 4 * H),  # o
        np.arange(2 * H, 3 * H),  # g
    ])
    out = {}

    def tile_k(a, nk):
        # [nk*128, M] -> [128, nk, M]
        return np.ascontiguousarray(
            a.reshape(nk, 128, a.shape[1]).transpose(1, 0, 2))

    def tile_km(a, nk):
        # [nk*128, 16*128] -> [16, 128, nk, 128]  (per-m-block contiguous)
        m = a.shape[1] // 128
        return np.ascontiguousarray(
            a.reshape(nk, 128, m, 128).transpose(2, 1, 0, 3))

    for d in ("f", "b"):
        # layer 0
        wih = inp[f"Wih_l0{d}"][perm]                     # [2048, 770]
        bias = (inp[f"bih_l0{d}"] + inp[f"bhh_l0{d}"])[perm]
        ext = np.zeros((K0 * 128, G), np.float32)
        ext[:768] = wih.T[:768]
        ext[768] = wih.T[768]      # delta coeffs at tile6 partition 0
        ext[800] = wih.T[769]      # mask coeffs at tile6 partition 32
        ext[832] = bias            # bias row at tile6 partition 64
        out[f"wih0{d}"] = _bf(tile_km(ext, K0))           # [16,128,7,128]
        whh = inp[f"Whh_l0{d}"][perm]                     # [2048, 512]
        out[f"whh0{d}"] = _bf(tile_k(whh.T, KH))          # [128, 4, 2048]
        # layer 1
        wih = inp[f"Wih_l1{d}"][perm]                     # [2048, 1024]
        bias = (inp[f"bih_l1{d}"] + inp[f"bhh_l1{d}"])[perm]
        ext = np.zeros((K1 * 128, G), np.float32)
        ext[:1024] = wih.T
        ext[1024] = bias
        out[f"wih1{d}"] = _bf(tile_km(ext, K1))           # [16,128,9,128]
        whh = inp[f"Whh_l1{d}"][perm]
        out[f"whh1{d}"] = _bf(tile_k(whh.T, KH))
    # output projection [1152, 30] with bias row at 1024
    ext = np.zeros((K1 * 128, C), np.float32)
    ext[:1024] = inp["W_out"].T
    ext[1024] = inp["b_out"]
    out["wout"] = _bf(tile_k(ext, K1))                    # [128, 9, 30]
    return out


def build_nc():
    nc = bacc.Bacc("TRN2", target_bir_lowering=False, debug=False,
                   num_devices=NCORES)
    hs = nc.dram_tensor("hs", [4, BL, S, E], BF16, kind="ExternalInput").ap()
    roles = nc.dram_tensor("roles", [1, SB], F32, kind="ExternalInput").ap()
    preds = nc.dram_tensor("preds", [1, SB], F32, kind="ExternalInput").ap()
    w = {}
    for d in ("f", "b"):
        w[f"wih0{d}"] = nc.dram_tensor(f"wih0{d}", [MT, 128, K0, 128], BF16,
                                       kind="ExternalInput").ap()
        w[f"wih1{d}"] = nc.dram_tensor(f"wih1{d}", [MT, 128, K1, 128], BF16,
                                       kind="ExternalInput").ap()
        w[f"whh0{d}"] = nc.dram_tensor(f"whh0{d}", [128, KH, G], BF16,
                                       kind="ExternalInput").ap()
        w[f"whh1{d}"] = nc.dram_tensor(f"whh1{d}", [128, KH, G], BF16,
                                       kind="ExternalInput").ap()
    w["wout"] = nc.dram_tensor("wout", [128, K1, C], BF16,
                               kind="ExternalInput").ap()
    out = nc.dram_tensor("out", [BL, S, C], F32, kind="ExternalOutput").ap()

    with tile.TileContext(nc) as tc:
        _emit(nc, tc, hs, roles, preds, w, out)
    nc.compile()
    return nc


def _emit(nc, tc, hs, roles, preds, w, out):
    from contextlib import ExitStack
    with ExitStack() as st:
        cpool = st.enter_context(tc.tile_pool(name="const", bufs=1))
        hlpool = st.enter_context(tc.tile_pool(name="hl", bufs=5))
        sumpool = st.enter_context(tc.tile_pool(name="sum", bufs=3))
        rpool = st.enter_context(tc.tile_pool(name="rows", bufs=1))
        xwpool = st.enter_context(tc.tile_pool(name="xw", bufs=2))
        scpool = st.enter_context(tc.tile_pool(name="sc", bufs=3))
        wpool = st.enter_context(tc.tile_pool(name="wts", bufs=2))
        pspool = st.enter_context(tc.tile_pool(name="ps", bufs=2, space="PSUM"))
        psg = st.enter_context(tc.tile_pool(name="psg", bufs=4, space="PSUM"))

        ident = cpool.tile([128, 128], F32, tag="ident")
        make_identity(nc, ident[:, :])
        ones_col = cpool.tile([128, 1], BF16, tag="onescol")
        nc.vector.memset(ones_col[:, :], 1.0)
        ones_row = cpool.tile([128, SB], BF16, tag="onesrow")
        nc.vector.memset(ones_row[:, :], 0.0)
        nc.vector.memset(ones_row[0:1, :], 1.0)

        # ---- x.T construction: [128, 7, SB] bf16 -------------------------
        xt = rpool.tile([128, K0, SB], BF16, tag="xt")
        hs_sbe = hs.rearrange("l b s e -> l s b e")
        for r in range(8):  # row-tiles of (t,b)
            acc = None
            hl = []
            for layer in range(4):
                t = hlpool.tile([128, E], BF16, tag="hl")
                nc.sync.dma_start(out=t[:, :],
                                  in_=hs_sbe[layer, 32 * r:32 * (r + 1), :, :])
                hl.append(t)
            s01 = sumpool.tile([128, E], F32, tag="sum")
            nc.vector.tensor_tensor(s01[:, :], hl[0][:, :], hl[1][:, :], OP.add)
            s23 = sumpool.tile([128, E], F32, tag="sum")
            nc.vector.tensor_tensor(s23[:, :], hl[2][:, :], hl[3][:, :], OP.add)
            ssum = sumpool.tile([128, E], F32, tag="sum")
            nc.vector.tensor_tensor(ssum[:, :], s01[:, :], s23[:, :], OP.add)
            for c in range(6):
                pt = pspool.tile([128, 128], F32, tag="tp")
                nc.tensor.transpose(pt[:, :], ssum[:, 128 * c:128 * (c + 1)],
                                    ident[:, :])
                nc.vector.tensor_scalar_mul(
                    xt[:, c, 128 * r:128 * (r + 1)], pt[:, :], 0.25)

        # ---- feature rows (delta, mask, ones) in xt[:, 6, :] -------------
        nc.vector.memset(xt[:, 6, :], 0.0)
        nc.vector.memset(xt[64:65, 6, :], 1.0)

        rrow = rpool.tile([1, SB], F32, tag="rrow")
        nc.sync.dma_start(out=rrow[:, :], in_=roles[:, :])
        prow = rpool.tile([1, SB], F32, tag="prow")
        nc.sync.dma_start(out=prow[:, :], in_=preds[:, :])
        m1 = rpool.tile([1, SB], F32, tag="m1")
        nc.vector.tensor_scalar(m1[:, :], rrow[:, :], 0.0, None, OP.not_equal)
        m2 = rpool.tile([1, SB], F32, tag="m2")
        nc.vector.tensor_scalar(m2[:, :], rrow[:, :], -100.0, None,
                                OP.not_equal)
        nc.vector.tensor_tensor(xt[32:33, 6, :], m1[:, :], m2[:, :], OP.mult)

        # mean_word row via ones-matmul over the 6 full e-tiles
        mw = rpool.tile([1, SB], F32, tag="mw")
        for ch in range(2):
            mp_ps = pspool.tile([1, 512], F32, tag="proj")
            for k in range(6):
                nc.tensor.matmul(mp_ps[:, :], ones_col[:, :],
                                 xt[:, k, 512 * ch:512 * (ch + 1)],
                                 start=(k == 0), stop=(k == 5))
            nc.vector.tensor_scalar_mul(mw[0:1, 512 * ch:512 * (ch + 1)],
                                        mp_ps[:, :], 1.0 / E)
        # first-predicate one-hot: oh = p * (cumsum(p) == 1)
        zrow = rpool.tile([1, SB], F32, tag="zrow")
        nc.vector.memset(zrow[:, :], 0.0)
        cs = rpool.tile([1, SB], F32, tag="cs")
        cs_b = cs.rearrange("p (t b) -> p b t", b=BL)
        pr_b = prow.rearrange("p (t b) -> p b t", b=BL)
        for b in range(BL):
            nc.vector.tensor_tensor_scan(cs_b[:, b, :], pr_b[:, b, :],
                                         zrow[0:1, 0:S], 0.0, OP.add, OP.add)
        oh = rpool.tile([1, SB], F32, tag="oh")
        nc.vector.tensor_scalar(oh[:, :], cs[:, :], 1.0, None, OP.is_equal)
        nc.vector.tensor_tensor(oh[:, :], oh[:, :], prow[:, :], OP.mult)
        nc.vector.tensor_tensor(oh[:, :], oh[:, :], mw[:, :], OP.mult)
        mpred = rpool.tile([1, BL], F32, tag="mpred")
        oh_b = oh.rearrange("p (t b) -> p b t", b=BL)
        nc.vector.tensor_reduce(mpred[:, :], oh_b[:, :, :],
                                mybir.AxisListType.X, OP.add)
        mw_b = mw.rearrange("p (t b) -> p b t", b=BL)
        xt6_b = xt.rearrange("p k (t b) -> p k b t", b=BL)
        for b in range(BL):
            nc.vector.tensor_scalar(xt6_b[0:1, 6, b, :], mw_b[:, b, :],
                                    mpred[0:1, b:b + 1], None, OP.subtract)

        # ---- projections + scans ----------------------------------------
        def projection(wih_dram, nk, rhs_of_k, xw):
            """xw[:, m, :] (bf16 [128, MT, SB]) = Wih_ext.T @ rhs (all t)."""
            for m in range(MT):
                wm = wpool.tile([128, nk, 128], BF16, tag="wihm")
                nc.sync.dma_start(out=wm[:, :, :], in_=wih_dram[m])
                for ch in range(2):
                    pp = pspool.tile([128, 512], F32, tag="proj")
                    for k in range(nk):
                        nc.tensor.matmul(pp[:, :], wm[:, k, :], rhs_of_k(k, ch),
                                         start=(k == 0), stop=(k == nk - 1))
                    nc.vector.tensor_copy(xw[:, m, 512 * ch:512 * (ch + 1)],
                                          pp[:, :])

        def scan_layer(whh_sb, xw, hdst):
            """Interleaved fwd/bwd 256-step scan. whh_sb/xw/hdst: dict d->tile"""
            hbuf, cbuf = {}, {}
            for d in ("f", "b"):
                hbuf[d] = rpool.tile([128, 2, KH, BL], BF16, tag=f"hbuf{d}", name=f"hbuf{d}")
                nc.vector.memset(hbuf[d][:, 0, :, :], 0.0)
                cbuf[d] = rpool.tile([128, KH, BL], F32, tag=f"cbuf{d}", name=f"cbuf{d}")
                nc.vector.memset(cbuf[d][:, :, :], 0.0)

            # xw viewed as [128, gate(4), mtile(4), SB] for half-slicing
            xwr = {d: xw[d].rearrange("p (g m) s -> p g m s", m=4)
                   for d in ("f", "b")}
            with tc.For_i(0, S, UNROLL, hint_engines=(mybir.EngineType.PE,),
                          staggered_reset=True) as i:
                for j in range(UNROLL):
                    for d in ("f", "b"):
                        cur, nxt = j % 2, (j + 1) % 2
                        if d == "f":
                            col = i * BL + j * BL
                        else:
                            col = i * (-BL) + (S - 1 - j) * BL
                        # two independent halves: half hh covers m-subtiles
                        # {2hh, 2hh+1} of each gate block -> h rows
                        # [256*hh, 256*(hh+1)), i.e. cbuf/hbuf k-tiles 2hh:2hh+2
                        for hh in range(2):
                            ph = psg.tile([128, 4, 2, BL], F32, tag="gates")
                            for g4 in range(4):
                                for mp in range(2):
                                    m = g4 * 4 + 2 * hh + mp
                                    for k in range(KH):
                                        nc.tensor.matmul(
                                            ph[:, g4, mp, :],
                                            whh_sb[d][:, k, 128 * m:128 * (m + 1)],
                                            hbuf[d][:, cur, k, :],
                                            start=(k == 0), stop=(k == KH - 1))
                            ks = slice(2 * hh, 2 * hh + 2)
                            gs = scpool.tile([128, 4, 2, BL], F32, tag="gsb")
                            nc.vector.tensor_tensor(
                                gs[:, :, :, :], ph[:, :, :, :],
                                xwr[d][:, :, ks, ds(col, BL)], OP.add)
                            sig = scpool.tile([128, 3, 2, BL], F32, tag="sig")
                            nc.scalar.activation(sig[:, :, :, :],
                                                 gs[:, 0:3, :, :], AF.Sigmoid)
                            tg = scpool.tile([128, 2, BL], F32, tag="tg")
                            nc.scalar.activation(tg[:, :, :], gs[:, 3, :, :],
                                                 AF.Tanh)
                            t1 = scpool.tile([128, 2, BL], F32, tag="t1")
                            nc.vector.tensor_tensor(t1[:, :, :], sig[:, 0, :, :],
                                                    tg[:, :, :], OP.mult)
                            t2 = scpool.tile([128, 2, BL], F32, tag="t2")
                            nc.vector.tensor_tensor(t2[:, :, :], sig[:, 1, :, :],
                                                    cbuf[d][:, ks, :], OP.mult)
                            nc.vector.tensor_tensor(cbuf[d][:, ks, :],
                                                    t1[:, :, :], t2[:, :, :],
                                                    OP.add)
                            tcc = scpool.tile([128, 2, BL], F32, tag="tcc")
                            nc.scalar.activation(tcc[:, :, :], cbuf[d][:, ks, :],
                                                 AF.Tanh)
                            nc.vector.tensor_tensor(hbuf[d][:, nxt, ks, :],
                                                    sig[:, 2, :, :],
                                                    tcc[:, :, :], OP.mult)
                        nc.vector.tensor_copy(hdst[d][:, :, ds(col, BL)],
                                              hbuf[d][:, nxt, :, :])

        # ---- layer 0 -----------------------------------------------------
        whh0 = {}
        for d in ("f", "b"):
            whh0[d] = wpool.tile([128, KH, G], BF16, tag="whh", name=f"whh0{d}")
            nc.sync.dma_start(out=whh0[d][:, :, :], in_=w[f"whh0{d}"][:, :, :])
        xw0 = {}
        for d in ("f", "b"):
            xw0[d] = xwpool.tile([128, MT, SB], BF16, tag="xw", name=f"xw0{d}")
            projection(w[f"wih0{d}"], K0,
                       lambda k, ch: xt[:, k, 512 * ch:512 * (ch + 1)],
                       xw0[d])
        h0 = {d: rpool.tile([128, KH, SB], BF16, tag=f"h0{d}", name=f"h0{d}")
              for d in ("f", "b")}
        scan_layer(whh0, xw0, h0)

        # ---- layer 1 -----------------------------------------------------
        whh1 = {}
        for d in ("f", "b"):
            whh1[d] = wpool.tile([128, KH, G], BF16, tag="whh", name=f"whh1{d}")
            nc.sync.dma_start(out=whh1[d][:, :, :], in_=w[f"whh1{d}"][:, :, :])

        def l1_rhs(k, ch):
            if k < KH:
                return h0["f"][:, k, 512 * ch:512 * (ch + 1)]
            if k < 2 * KH:
                return h0["b"][:, k - KH, 512 * ch:512 * (ch + 1)]
            return ones_row[:, 512 * ch:512 * (ch + 1)]

        xw1 = {}
        for d in ("f", "b"):
            xw1[d] = xwpool.tile([128, MT, SB], BF16, tag="xw", name=f"xw1{d}")
            projection(w[f"wih1{d}"], K1, l1_rhs, xw1[d])
        h1 = {d: rpool.tile([128, KH, SB], BF16, tag=f"h1{d}", name=f"h1{d}")
              for d in ("f", "b")}
        scan_layer(whh1, xw1, h1)

        # ---- output projection ------------------------------------------
        wo = wpool.tile([128, K1, C], BF16, tag="wout")
        nc.sync.dma_start(out=wo[:, :, :], in_=w["wout"][:, :, :])
        outT = rpool.tile([C, SB], F32, tag="outT")
        for ch in range(2):
            po = pspool.tile([C, 512], F32, tag="proj")
            for k in range(K1):
                if k < KH:
                    rhs = h1["f"][:, k, 512 * ch:512 * (ch + 1)]
                elif k < 2 * KH:
                    rhs = h1["b"][:, k - KH, 512 * ch:512 * (ch + 1)]
                else:
                    rhs = ones_row[:, 512 * ch:512 * (ch + 1)]
                nc.tensor.matmul(po[:, :], wo[:, k, :], rhs,
                                 start=(k == 0), stop=(k == K1 - 1))
            nc.vector.tensor_copy(outT[:, 512 * ch:512 * (ch + 1)], po[:, :])
        out_sbc = out.rearrange("b s c -> s b c")
        for cb in range(8):
            pt = pspool.tile([128, C], F32, tag="tp")
            nc.tensor.transpose(pt[:, :], outT[:, 128 * cb:128 * (cb + 1)],
                                ident[0:C, 0:C])
            onat = scpool.tile([128, C], F32, tag="onat")
            nc.vector.tensor_copy(onat[:, :], pt[:, :])
            nc.sync.dma_start(out=out_sbc[32 * cb:32 * (cb + 1), :, :],
                              in_=onat[:, :])


def _get_nc():
    if "nc" not in _cache:
        _cache["nc"] = build_nc()
    return _cache["nc"]


def kernel(**inputs):
    from concourse.bass_utils import run_bass_kernel_spmd

    wmaps = _prep_weights(inputs)
    hsf = np.asarray(inputs["hidden_states"], np.float32)
    rol = np.asarray(inputs["roles"])
    prd = np.asarray(inputs["predicates"])
    in_maps = []
    for c in range(NCORES):
        sl = slice(BL * c, BL * (c + 1))
        m = dict(wmaps)
        m["hs"] = _bf(hsf[:, sl])                                   # [4,BL,S,E]
        m["roles"] = np.ascontiguousarray(
            rol[sl].T.reshape(1, SB)).astype(np.float32)            # (t,b)
        m["preds"] = np.ascontiguousarray(
            prd[sl].T.reshape(1, SB)).astype(np.float32)
        in_maps.append(m)

    nc = _get_nc()
    res = run_bass_kernel_spmd(nc, in_maps, core_ids=list(range(NCORES)))
    return np.concatenate([r["out"] for r in res.results], axis=0)


# revision 9
# speedup vs baseline: 1.1525x; 1.1525x over previous
"""Trainium2 Bass kernel for nn_ArgumentClassification (2-layer BiLSTM tagger).

Sharding: data-parallel over batch B=32 across 8 NeuronCores (4 rows each),
LSTM/Linear weights replicated. No collectives.

Per-core pipeline (all compute on device):
  1. mean over 4 transformer layers of hidden_states  -> x [4,256,768]
  2. predicate-relative delta + role mask features    -> x_ext [4,256,770]
     (x is built directly in transposed layout x.T [770, S*B] via PE transposes)
  3. L0 BiLSTM: input projection (batched matmul over all timesteps, biases
     folded in via a ones-row), then the sequential 256-step scan in
     gates-transposed layout [2048, B] with Whh stationary on the PE.
     Forward/backward directions interleaved so the PE never waits on the
     DVE/ACT gate-nonlinearity tail.
  4. L1 BiLSTM: same, input = [h0f; h0b].
  5. out = h1 @ W_out.T + b_out, PE-transposed back to [B,S,30] and DMA'd out.

Gate order is host-permuted from PyTorch's (i,f,g,o) to (i,f,o,g) so the scan
needs only two activation instructions per step: sigmoid over tiles 0:12 and
tanh over tiles 12:16.
"""
import sys

sys.path.insert(0, "/opt/trn_rl_repo")

import numpy as np
import ml_dtypes

import concourse.bass as bass
import concourse.tile as tile
from concourse import bacc, mybir
from concourse.bass import ds
from concourse.masks import make_identity

BF16 = mybir.dt.bfloat16
F32 = mybir.dt.float32
AF = mybir.ActivationFunctionType
OP = mybir.AluOpType

B, S, E, H, C = 32, 256, 768, 512, 30
NCORES = 8
BL = B // NCORES          # 4 rows per core
SB = S * BL               # 1024 columns, ordered (t, b): col = t*BL + b
G = 4 * H                 # 2048 gate rows
MT = G // 128             # 16 gate m-tiles
KH = H // 128             # 4 hidden k-tiles
K0 = 7                    # L0 input k-tiles ([770 + ones-row] padded to 896)
K1 = 9                    # L1 input k-tiles (1024 + ones-row -> 1152)
UNROLL = 32

_cache = {}


def _bf(a):
    return np.asarray(a, dtype=ml_dtypes.bfloat16)


def _prep_weights(inp):
    """Host-side: permute gates to (i,f,o,g), transpose, pad, fold biases,
    tile for SBUF. Returns dict of name -> np array matching dram params."""
    perm = np.concatenate([
        np.arange(0, H),          # i
        np.arange(H, 2 * H),      # f
        np.arange(3 * H, 4 * H),  # o
        np.arange(2 * H, 3 * H),  # g
    ])
    out = {}

    def tile_k(a, nk):
        # [nk*128, M] -> [128, nk, M]
        return np.ascontiguousarray(
            a.reshape(nk, 128, a.shape[1]).transpose(1, 0, 2))

    def tile_km(a, nk):
        # [nk*128, 16*128] -> [16, 128, nk, 128]  (per-m-block contiguous)
        m = a.shape[1] // 128
        return np.ascontiguousarray(
            a.reshape(nk, 128, m, 128).transpose(2, 1, 0, 3))

    for d in ("f", "b"):
        # layer 0
        wih = inp[f"Wih_l0{d}"][perm]                     # [2048, 770]
        bias = (inp[f"bih_l0{d}"] + inp[f"bhh_l0{d}"])[perm]
        ext = np.zeros((K0 * 128, G), np.float32)
        ext[:768] = wih.T[:768]
        ext[768] = wih.T[768]      # delta coeffs at tile6 partition 0
        ext[800] = wih.T[769]      # mask coeffs at tile6 partition 32
        ext[832] = bias            # bias row at tile6 partition 64
        out[f"wih0{d}"] = _bf(tile_km(ext, K0))           # [16,128,7,128]
        whh = inp[f"Whh_l0{d}"][perm]                     # [2048, 512]
        out[f"whh0{d}"] = _bf(tile_k(whh.T, KH))          # [128, 4, 2048]
        # layer 1
        wih = inp[f"Wih_l1{d}"][perm]                     # [2048, 1024]
        bias = (inp[f"bih_l1{d}"] + inp[f"bhh_l1{d}"])[perm]
        ext = np.zeros((K1 * 128, G), np.float32)
        ext[:1024] = wih.T
        ext[1024] = bias
        out[f"wih1{d}"] = _bf(tile_km(ext, K1))           # [16,128,9,128]
        whh = inp[f"Whh_l1{d}"][perm]
        out[f"whh1{d}"] = _bf(tile_k(whh.T, KH))
    # output projection [1152, 30] with bias row at 1024
    ext = np.zeros((K1 * 128, C), np.float32)
    ext[:1024] = inp["W_out"].T
    ext[1024] = inp["b_out"]
    out["wout"] = _bf(tile_k(ext, K1))                    # [128, 9, 30]
    return out


def build_nc():
    nc = bacc.Bacc("TRN2", target_bir_lowering=False, debug=False,
                   num_devices=NCORES)
    hs = nc.dram_tensor("hs", [4, BL, S, E], BF16, kind="ExternalInput").ap()
    roles = nc.dram_tensor("roles", [1, SB], F32, kind="ExternalInput").ap()
    preds = nc.dram_tensor("preds", [1, SB], F32, kind="ExternalInput").ap()
    w = {}
    for d in ("f", "b"):
        w[f"wih0{d}"] = nc.dram_tensor(f"wih0{d}", [MT, 128, K0, 128], BF16,
                                       kind="ExternalInput").ap()
        w[f"wih1{d}"] = nc.dram_tensor(f"wih1{d}", [MT, 128, K1, 128], BF16,
                                       kind="ExternalInput").ap()
        w[f"whh0{d}"] = nc.dram_tensor(f"whh0{d}", [128, KH, G], BF16,
                                       kind="ExternalInput").ap()
        w[f"whh1{d}"] = nc.dram_tensor(f"whh1{d}", [128, KH, G], BF16,
                                       kind="ExternalInput").ap()
    w["wout"] = nc.dram_tensor("wout", [128, K1, C], BF16,
                               kind="ExternalInput").ap()
    out = nc.dram_tensor("out", [BL, S, C], F32, kind="ExternalOutput").ap()

    with tile.TileContext(nc) as tc:
        _emit(nc, tc, hs, roles, preds, w, out)
    nc.compile()
    return nc


def _emit(nc, tc, hs, roles, preds, w, out):
    from contextlib import ExitStack
    with ExitStack() as st:
        cpool = st.enter_context(tc.tile_pool(name="const", bufs=1))
        hlpool = st.enter_context(tc.tile_pool(name="hl", bufs=5))
        sumpool = st.enter_context(tc.tile_pool(name="sum", bufs=3))
        rpool = st.enter_context(tc.tile_pool(name="rows", bufs=1))
        xwpool = st.enter_context(tc.tile_pool(name="xw", bufs=2))
        scpool = st.enter_context(tc.tile_pool(name="sc", bufs=3))
        wpool = st.enter_context(tc.tile_pool(name="wts", bufs=2))
        pspool = st.enter_context(tc.tile_pool(name="ps", bufs=2, space="PSUM"))
        psg = st.enter_context(tc.tile_pool(name="psg", bufs=3, space="PSUM"))

        ident = cpool.tile([128, 128], F32, tag="ident")
        make_identity(nc, ident[:, :])
        ones_col = cpool.tile([128, 1], BF16, tag="onescol")
        nc.vector.memset(ones_col[:, :], 1.0)
        ones_row = cpool.tile([128, SB], BF16, tag="onesrow")
        nc.vector.memset(ones_row[:, :], 0.0)
        nc.vector.memset(ones_row[0:1, :], 1.0)

        # ---- x.T construction: [128, 7, SB] bf16 -------------------------
        xt = rpool.tile([128, K0, SB], BF16, tag="xt")
        hs_sbe = hs.rearrange("l b s e -> l s b e")
        for r in range(8):  # row-tiles of (t,b)
            acc = None
            hl = []
            for layer in range(4):
                t = hlpool.tile([128, E], BF16, tag="hl")
                nc.sync.dma_start(out=t[:, :],
                                  in_=hs_sbe[layer, 32 * r:32 * (r + 1), :, :])
                hl.append(t)
            s01 = sumpool.tile([128, E], F32, tag="sum")
            nc.vector.tensor_tensor(s01[:, :], hl[0][:, :], hl[1][:, :], OP.add)
            s23 = sumpool.tile([128, E], F32, tag="sum")
            nc.vector.tensor_tensor(s23[:, :], hl[2][:, :], hl[3][:, :], OP.add)
            ssum = sumpool.tile([128, E], F32, tag="sum")
            nc.vector.tensor_tensor(ssum[:, :], s01[:, :], s23[:, :], OP.add)
            for c in range(6):
                pt = pspool.tile([128, 128], F32, tag="tp")
                nc.tensor.transpose(pt[:, :], ssum[:, 128 * c:128 * (c + 1)],
                                    ident[:, :])
                nc.vector.tensor_scalar_mul(
                    xt[:, c, 128 * r:128 * (r + 1)], pt[:, :], 0.25)

        # ---- feature rows (delta, mask, ones) in xt[:, 6, :] -------------
        nc.vector.memset(xt[:, 6, :], 0.0)
        nc.vector.memset(xt[64:65, 6, :], 1.0)

        rrow = rpool.tile([1, SB], F32, tag="rrow")
        nc.sync.dma_start(out=rrow[:, :], in_=roles[:, :])
        prow = rpool.tile([1, SB], F32, tag="prow")
        nc.sync.dma_start(out=prow[:, :], in_=preds[:, :])
        m1 = rpool.tile([1, SB], F32, tag="m1")
        nc.vector.tensor_scalar(m1[:, :], rrow[:, :], 0.0, None, OP.not_equal)
        m2 = rpool.tile([1, SB], F32, tag="m2")
        nc.vector.tensor_scalar(m2[:, :], rrow[:, :], -100.0, None,
                                OP.not_equal)
        nc.vector.tensor_tensor(xt[32:33, 6, :], m1[:, :], m2[:, :], OP.mult)

        # mean_word row via ones-matmul over the 6 full e-tiles
        mw = rpool.tile([1, SB], F32, tag="mw")
        for ch in range(2):
            mp_ps = pspool.tile([1, 512], F32, tag="proj")
            for k in range(6):
                nc.tensor.matmul(mp_ps[:, :], ones_col[:, :],
                                 xt[:, k, 512 * ch:512 * (ch + 1)],
                                 start=(k == 0), stop=(k == 5))
            nc.vector.tensor_scalar_mul(mw[0:1, 512 * ch:512 * (ch + 1)],
                                        mp_ps[:, :], 1.0 / E)
        # first-predicate one-hot: oh = p * (cumsum(p) == 1)
        zrow = rpool.tile([1, SB], F32, tag="zrow")
        nc.vector.memset(zrow[:, :], 0.0)
        cs = rpool.tile([1, SB], F32, tag="cs")
        cs_b = cs.rearrange("p (t b) -> p b t", b=BL)
        pr_b = prow.rearrange("p (t b) -> p b t", b=BL)
        for b in range(BL):
            nc.vector.tensor_tensor_scan(cs_b[:, b, :], pr_b[:, b, :],
                                         zrow[0:1, 0:S], 0.0, OP.add, OP.add)
        oh = rpool.tile([1, SB], F32, tag="oh")
        nc.vector.tensor_scalar(oh[:, :], cs[:, :], 1.0, None, OP.is_equal)
        nc.vector.tensor_tensor(oh[:, :], oh[:, :], prow[:, :], OP.mult)
        nc.vector.tensor_tensor(oh[:, :], oh[:, :], mw[:, :], OP.mult)
        mpred = rpool.tile([1, BL], F32, tag="mpred")
        oh_b = oh.rearrange("p (t b) -> p b t", b=BL)
        nc.vector.tensor_reduce(mpred[:, :], oh_b[:, :, :],
                                mybir.AxisListType.X, OP.add)
        mw_b = mw.rearrange("p (t b) -> p b t", b=BL)
        xt6_b = xt.rearrange("p k (t b) -> p k b t", b=BL)
        for b in range(BL):
            nc.vector.tensor_scalar(xt6_b[0:1, 6, b, :], mw_b[:, b, :],
                                    mpred[0:1, b:b + 1], None, OP.subtract)

        # ---- projections + scans ----------------------------------------
        def projection(wih_dram, nk, rhs_of_k, xw):
            """xw[:, m, :] (bf16 [128, MT, SB]) = Wih_ext.T @ rhs (all t)."""
            for m in range(MT):
                wm = wpool.tile([128, nk, 128], BF16, tag="wihm")
                nc.sync.dma_start(out=wm[:, :, :], in_=wih_dram[m])
                for ch in range(2):
                    pp = pspool.tile([128, 512], F32, tag="proj")
                    for k in range(nk):
                        nc.tensor.matmul(pp[:, :], wm[:, k, :], rhs_of_k(k, ch),
                                         start=(k == 0), stop=(k == nk - 1))
                    nc.vector.tensor_copy(xw[:, m, 512 * ch:512 * (ch + 1)],
                                          pp[:, :])

        def scan_layer(whh_sb, xw, hdst):
            """Interleaved fwd/bwd 256-step scan. whh_sb/xw/hdst: dict d->tile"""
            hbuf, cbuf = {}, {}
            for d in ("f", "b"):
                hbuf[d] = rpool.tile([128, 2, KH, BL], BF16, tag=f"hbuf{d}", name=f"hbuf{d}")
                nc.vector.memset(hbuf[d][:, 0, :, :], 0.0)
                cbuf[d] = rpool.tile([128, KH, BL], F32, tag=f"cbuf{d}", name=f"cbuf{d}")
                nc.vector.memset(cbuf[d][:, :, :], 0.0)

            with tc.For_i(0, S, UNROLL, hint_engines=(mybir.EngineType.PE,)) as i:
                for j in range(UNROLL):
                    for d in ("f", "b"):
                        cur, nxt = j % 2, (j + 1) % 2
                        if d == "f":
                            col = i * BL + j * BL
                        else:
                            col = i * (-BL) + (S - 1 - j) * BL
                        ps = psg.tile([128, MT, BL], F32, tag="gates")
                        for m in range(MT):
                            for k in range(KH):
                                nc.tensor.matmul(
                                    ps[:, m, :],
                                    whh_sb[d][:, k, 128 * m:128 * (m + 1)],
                                    hbuf[d][:, cur, k, :],
                                    start=(k == 0), stop=(k == KH - 1))
                        gsb = scpool.tile([128, MT, BL], F32, tag="gsb")
                        nc.vector.tensor_tensor(
                            gsb[:, :, :], ps[:, :, :],
                            xw[d][:, :, ds(col, BL)], OP.add)
                        sig = scpool.tile([128, 12, BL], F32, tag="sig")
                        nc.scalar.activation(sig[:, :, :], gsb[:, 0:12, :],
                                             AF.Sigmoid)
                        tg = scpool.tile([128, KH, BL], F32, tag="tg")
                        nc.scalar.activation(tg[:, :, :], gsb[:, 12:16, :],
                                             AF.Tanh)
                        t1 = scpool.tile([128, KH, BL], F32, tag="t1")
                        nc.vector.tensor_tensor(t1[:, :, :], sig[:, 0:4, :],
                                                tg[:, :, :], OP.mult)
                        t2 = scpool.tile([128, KH, BL], F32, tag="t2")
                        nc.vector.tensor_tensor(t2[:, :, :], sig[:, 4:8, :],
                                                cbuf[d][:, :, :], OP.mult)
                        nc.vector.tensor_tensor(cbuf[d][:, :, :], t1[:, :, :],
                                                t2[:, :, :], OP.add)
                        tcc = scpool.tile([128, KH, BL], F32, tag="tcc")
                        nc.scalar.activation(tcc[:, :, :], cbuf[d][:, :, :],
                                             AF.Tanh)
                        nc.vector.tensor_tensor(hbuf[d][:, nxt, :, :],
                                                sig[:, 8:12, :], tcc[:, :, :],
                                                OP.mult)
                        nc.vector.tensor_copy(hdst[d][:, :, ds(col, BL)],
                                              hbuf[d][:, nxt, :, :])

        # ---- layer 0 -----------------------------------------------------
        whh0 = {}
        for d in ("f", "b"):
            whh0[d] = wpool.tile([128, KH, G], BF16, tag="whh", name=f"whh0{d}")
            nc.sync.dma_start(out=whh0[d][:, :, :], in_=w[f"whh0{d}"][:, :, :])
        xw0 = {}
        for d in ("f", "b"):
            xw0[d] = xwpool.tile([128, MT, SB], BF16, tag="xw", name=f"xw0{d}")
            projection(w[f"wih0{d}"], K0,
                       lambda k, ch: xt[:, k, 512 * ch:512 * (ch + 1)],
                       xw0[d])
        h0 = {d: rpool.tile([128, KH, SB], BF16, tag=f"h0{d}", name=f"h0{d}")
              for d in ("f", "b")}
        scan_layer(whh0, xw0, h0)

        # ---- layer 1 -----------------------------------------------------
        whh1 = {}
        for d in ("f", "b"):
            whh1[d] = wpool.tile([128, KH, G], BF16, tag="whh", name=f"whh1{d}")
            nc.sync.dma_start(out=whh1[d][:, :, :], in_=w[f"whh1{d}"][:, :, :])

        def l1_rhs(k, ch):
            if k < KH:
                return h0["f"][:, k, 512 * ch:512 * (ch + 1)]
            if k < 2 * KH:
                return h0["b"][:, k - KH, 512 * ch:512 * (ch + 1)]
            return ones_row[:, 512 * ch:512 * (ch + 1)]

        xw1 = {}
        for d in ("f", "b"):
            xw1[d] = xwpool.tile([128, MT, SB], BF16, tag="xw", name=f"xw1{d}")
            projection(w[f"wih1{d}"], K1, l1_rhs, xw1[d])
        h1 = {d: rpool.tile([128, KH, SB], BF16, tag=f"h1{d}", name=f"h1{d}")
              for d in ("f", "b")}
        scan_layer(whh1, xw1, h1)

        # ---- output projection ------------------------------------------
        wo = wpool.tile([128, K1, C], BF16, tag="wout")
        nc.sync.dma_start(out=wo[:, :, :], in_=w["wout"][:, :, :])
        outT = rpool.tile([C, SB], F32, tag="outT")
        for ch in range(2):
            po = pspool.tile([C, 512], F32, tag="proj")
            for k in range(K1):
                if k < KH:
                    rhs = h1["f"][:, k, 512 * ch:512 * (ch + 1)]
                elif k < 2 * KH:
                    rhs = h1["b"][:, k - KH, 512 * ch:512 * (ch + 1)]
                else:
                    rhs = ones_row[:, 512 * ch:512 * (ch + 1)]
                nc.tensor.matmul(po[:, :], wo[:, k, :], rhs,
                                 start=(k == 0), stop=(k == K1 - 1))
            nc.vector.tensor_copy(outT[:, 512 * ch:512 * (ch + 1)], po[:, :])
        out_sbc = out.rearrange("b s c -> s b c")
        for cb in range(8):
            pt = pspool.tile([128, C], F32, tag="tp")
            nc.tensor.transpose(pt[:, :], outT[:, 128 * cb:128 * (cb + 1)],
                                ident[0:C, 0:C])
            onat = scpool.tile([128, C], F32, tag="onat")
            nc.vector.tensor_copy(onat[:, :], pt[:, :])
            nc.sync.dma_start(out=out_sbc[32 * cb:32 * (cb + 1), :, :],
                              in_=onat[:, :])


def _get_nc():
    if "nc" not in _cache:
        _cache["nc"] = build_nc()
    return _cache["nc"]


def kernel(**inputs):
    from concourse.bass_utils import run_bass_kernel_spmd

    wmaps = _prep_weights(inputs)
    hsf = np.asarray(inputs["hidden_states"], np.float32)
    rol = np.asarray(inputs["roles"])
    prd = np.asarray(inputs["predicates"])
    in_maps = []
    for c in range(NCORES):
        sl = slice(BL * c, BL * (c + 1))
        m = dict(wmaps)
        m["hs"] = _bf(hsf[:, sl])                                   # [4,BL,S,E]
        m["roles"] = np.ascontiguousarray(
            rol[sl].T.reshape(1, SB)).astype(np.float32)            # (t,b)
        m["preds"] = np.ascontiguousarray(
            prd[sl].T.reshape(1, SB)).astype(np.float32)
        in_maps.append(m)

    nc = _get_nc()
    res = run_bass_kernel_spmd(nc, in_maps, core_ids=list(range(NCORES)))
    return np.concatenate([r["out"] for r in res.results], axis=0)


# revision 10
# speedup vs baseline: 1.3144x; 1.1405x over previous
"""Trainium2 Bass kernel for nn_ArgumentClassification (2-layer BiLSTM tagger).

Sharding: data-parallel over batch B=32 across 8 NeuronCores (4 rows each),
LSTM/Linear weights replicated. No collectives.

Per-core pipeline (all compute on device):
  1. mean over 4 transformer layers of hidden_states  -> x [4,256,768]
  2. predicate-relative delta + role mask features    -> x_ext [4,256,770]
     (x is built directly in transposed layout x.T [770, S*B] via PE transposes)
  3. L0 BiLSTM: input projection (batched matmul over all timesteps, biases
     folded in via a ones-row), then the sequential 256-step scan in
     gates-transposed layout [2048, B] with Whh stationary on the PE.
     Forward/backward directions interleaved so the PE never waits on the
     DVE/ACT gate-nonlinearity tail.
  4. L1 BiLSTM: same, input = [h0f; h0b].
  5. out = h1 @ W_out.T + b_out, PE-transposed back to [B,S,30] and DMA'd out.

Gate order is host-permuted from PyTorch's (i,f,g,o) to (i,f,o,g) so the scan
needs only two activation instructions per step: sigmoid over tiles 0:12 and
tanh over tiles 12:16.
"""
import sys

sys.path.insert(0, "/opt/trn_rl_repo")

import numpy as np
import ml_dtypes

import concourse.bass as bass
import concourse.tile as tile
from concourse import bacc, mybir
from concourse.bass import ds
from concourse.masks import make_identity

BF16 = mybir.dt.bfloat16
F32 = mybir.dt.float32
AF = mybir.ActivationFunctionType
OP = mybir.AluOpType

B, S, E, H, C = 32, 256, 768, 512, 30
NCORES = 8
BL = B // NCORES          # 4 rows per core
SB = S * BL               # 1024 columns, ordered (t, b): col = t*BL + b
G = 4 * H                 # 2048 gate rows
MT = G // 128             # 16 gate m-tiles
KH = H // 128             # 4 hidden k-tiles
K0 = 7                    # L0 input k-tiles ([770 + ones-row] padded to 896)
K1 = 9                    # L1 input k-tiles (1024 + ones-row -> 1152)
UNROLL = 8

_cache = {}


def _bf(a):
    return np.asarray(a, dtype=ml_dtypes.bfloat16)


def _prep_weights(inp):
    """Host-side: permute gates to (i,f,o,g), transpose, pad, fold biases,
    tile for SBUF. Returns dict of name -> np array matching dram params."""
    perm = np.concatenate([
        np.arange(0, H),          # i
        np.arange(H, 2 * H),      # f
        np.arange(3 * H, 4 * H),  # o
        np.arange(2 * H, 3 * H),  # g
    ])
    out = {}

    def tile_k(a, nk):
        # [nk*128, M] -> [128, nk, M]
        return np.ascontiguousarray(
            a.reshape(nk, 128, a.shape[1]).transpose(1, 0, 2))

    def tile_km(a, nk):
        # [nk*128, 16*128] -> [16, 128, nk, 128]  (per-m-block contiguous)
        m = a.shape[1] // 128
        return np.ascontiguousarray(
            a.reshape(nk, 128, m, 128).transpose(2, 1, 0, 3))

    for d in ("f", "b"):
        # layer 0
        wih = inp[f"Wih_l0{d}"][perm]                     # [2048, 770]
        bias = (inp[f"bih_l0{d}"] + inp[f"bhh_l0{d}"])[perm]
        ext = np.zeros((K0 * 128, G), np.float32)
        ext[:768] = wih.T[:768]
        ext[768] = wih.T[768]      # delta coeffs at tile6 partition 0
        ext[800] = wih.T[769]      # mask coeffs at tile6 partition 32
        ext[832] = bias            # bias row at tile6 partition 64
        out[f"wih0{d}"] = _bf(tile_km(ext, K0))           # [16,128,7,128]
        whh = inp[f"Whh_l0{d}"][perm]                     # [2048, 512]
        out[f"whh0{d}"] = _bf(tile_k(whh.T, KH))          # [128, 4, 2048]
        # layer 1
        wih = inp[f"Wih_l1{d}"][perm]                     # [2048, 1024]
        bias = (inp[f"bih_l1{d}"] + inp[f"bhh_l1{d}"])[perm]
        ext = np.zeros((K1 * 128, G), np.float32)
        ext[:1024] = wih.T
        ext[1024] = bias
        out[f"wih1{d}"] = _bf(tile_km(ext, K1))           # [16,128,9,128]
        whh = inp[f"Whh_l1{d}"][perm]
        out[f"whh1{d}"] = _bf(tile_k(whh.T, KH))
    # output projection [1152, 30] with bias row at 1024
    ext = np.zeros((K1 * 128, C), np.float32)
    ext[:1024] = inp["W_out"].T
    ext[1024] = inp["b_out"]
    out["wout"] = _bf(tile_k(ext, K1))                    # [128, 9, 30]
    return out


def build_nc():
    nc = bacc.Bacc("TRN2", target_bir_lowering=False, debug=False,
                   num_devices=NCORES)
    hs = nc.dram_tensor("hs", [4, BL, S, E], BF16, kind="ExternalInput").ap()
    roles = nc.dram_tensor("roles", [1, SB], F32, kind="ExternalInput").ap()
    preds = nc.dram_tensor("preds", [1, SB], F32, kind="ExternalInput").ap()
    w = {}
    for d in ("f", "b"):
        w[f"wih0{d}"] = nc.dram_tensor(f"wih0{d}", [MT, 128, K0, 128], BF16,
                                       kind="ExternalInput").ap()
        w[f"wih1{d}"] = nc.dram_tensor(f"wih1{d}", [MT, 128, K1, 128], BF16,
                                       kind="ExternalInput").ap()
        w[f"whh0{d}"] = nc.dram_tensor(f"whh0{d}", [128, KH, G], BF16,
                                       kind="ExternalInput").ap()
        w[f"whh1{d}"] = nc.dram_tensor(f"whh1{d}", [128, KH, G], BF16,
                                       kind="ExternalInput").ap()
    w["wout"] = nc.dram_tensor("wout", [128, K1, C], BF16,
                               kind="ExternalInput").ap()
    out = nc.dram_tensor("out", [BL, S, C], F32, kind="ExternalOutput").ap()

    with tile.TileContext(nc) as tc:
        _emit(nc, tc, hs, roles, preds, w, out)
    nc.compile()
    return nc


def _emit(nc, tc, hs, roles, preds, w, out):
    from contextlib import ExitStack
    with ExitStack() as st:
        cpool = st.enter_context(tc.tile_pool(name="const", bufs=1))
        hlpool = st.enter_context(tc.tile_pool(name="hl", bufs=5))
        sumpool = st.enter_context(tc.tile_pool(name="sum", bufs=3))
        rpool = st.enter_context(tc.tile_pool(name="rows", bufs=1))
        xwpool = st.enter_context(tc.tile_pool(name="xw", bufs=2))
        scpool = st.enter_context(tc.tile_pool(name="sc", bufs=3))
        wpool = st.enter_context(tc.tile_pool(name="wts", bufs=2))
        pspool = st.enter_context(tc.tile_pool(name="ps", bufs=1, space="PSUM"))
        psg = st.enter_context(tc.tile_pool(name="psg", bufs=6, space="PSUM"))

        ident = cpool.tile([128, 128], F32, tag="ident")
        make_identity(nc, ident[:, :])
        ones_col = cpool.tile([128, 1], BF16, tag="onescol")
        nc.vector.memset(ones_col[:, :], 1.0)
        ones_row = cpool.tile([128, SB], BF16, tag="onesrow")
        nc.vector.memset(ones_row[:, :], 0.0)
        nc.vector.memset(ones_row[0:1, :], 1.0)

        # ---- x.T construction: [128, 7, SB] bf16 -------------------------
        xt = rpool.tile([128, K0, SB], BF16, tag="xt")
        hs_sbe = hs.rearrange("l b s e -> l s b e")
        for r in range(8):  # row-tiles of (t,b)
            acc = None
            hl = []
            for layer in range(4):
                t = hlpool.tile([128, E], BF16, tag="hl")
                nc.sync.dma_start(out=t[:, :],
                                  in_=hs_sbe[layer, 32 * r:32 * (r + 1), :, :])
                hl.append(t)
            s01 = sumpool.tile([128, E], F32, tag="sum")
            nc.vector.tensor_tensor(s01[:, :], hl[0][:, :], hl[1][:, :], OP.add)
            s23 = sumpool.tile([128, E], F32, tag="sum")
            nc.vector.tensor_tensor(s23[:, :], hl[2][:, :], hl[3][:, :], OP.add)
            ssum = sumpool.tile([128, E], F32, tag="sum")
            nc.vector.tensor_tensor(ssum[:, :], s01[:, :], s23[:, :], OP.add)
            for c in range(6):
                pt = pspool.tile([128, 128], F32, tag="tp")
                nc.tensor.transpose(pt[:, :], ssum[:, 128 * c:128 * (c + 1)],
                                    ident[:, :])
                nc.vector.tensor_scalar_mul(
                    xt[:, c, 128 * r:128 * (r + 1)], pt[:, :], 0.25)

        # ---- feature rows (delta, mask, ones) in xt[:, 6, :] -------------
        nc.vector.memset(xt[:, 6, :], 0.0)
        nc.vector.memset(xt[64:65, 6, :], 1.0)

        rrow = rpool.tile([1, SB], F32, tag="rrow")
        nc.sync.dma_start(out=rrow[:, :], in_=roles[:, :])
        prow = rpool.tile([1, SB], F32, tag="prow")
        nc.sync.dma_start(out=prow[:, :], in_=preds[:, :])
        m1 = rpool.tile([1, SB], F32, tag="m1")
        nc.vector.tensor_scalar(m1[:, :], rrow[:, :], 0.0, None, OP.not_equal)
        m2 = rpool.tile([1, SB], F32, tag="m2")
        nc.vector.tensor_scalar(m2[:, :], rrow[:, :], -100.0, None,
                                OP.not_equal)
        nc.vector.tensor_tensor(xt[32:33, 6, :], m1[:, :], m2[:, :], OP.mult)

        # mean_word row via ones-matmul over the 6 full e-tiles
        mw = rpool.tile([1, SB], F32, tag="mw")
        for ch in range(2):
            mp_ps = pspool.tile([1, 512], F32, tag="proj")
            for k in range(6):
                nc.tensor.matmul(mp_ps[:, :], ones_col[:, :],
                                 xt[:, k, 512 * ch:512 * (ch + 1)],
                                 start=(k == 0), stop=(k == 5))
            nc.vector.tensor_scalar_mul(mw[0:1, 512 * ch:512 * (ch + 1)],
                                        mp_ps[:, :], 1.0 / E)
        # first-predicate one-hot: oh = p * (cumsum(p) == 1)
        zrow = rpool.tile([1, SB], F32, tag="zrow")
        nc.vector.memset(zrow[:, :], 0.0)
        cs = rpool.tile([1, SB], F32, tag="cs")
        cs_b = cs.rearrange("p (t b) -> p b t", b=BL)
        pr_b = prow.rearrange("p (t b) -> p b t", b=BL)
        for b in range(BL):
            nc.vector.tensor_tensor_scan(cs_b[:, b, :], pr_b[:, b, :],
                                         zrow[0:1, 0:S], 0.0, OP.add, OP.add)
        oh = rpool.tile([1, SB], F32, tag="oh")
        nc.vector.tensor_scalar(oh[:, :], cs[:, :], 1.0, None, OP.is_equal)
        nc.vector.tensor_tensor(oh[:, :], oh[:, :], prow[:, :], OP.mult)
        nc.vector.tensor_tensor(oh[:, :], oh[:, :], mw[:, :], OP.mult)
        mpred = rpool.tile([1, BL], F32, tag="mpred")
        oh_b = oh.rearrange("p (t b) -> p b t", b=BL)
        nc.vector.tensor_reduce(mpred[:, :], oh_b[:, :, :],
                                mybir.AxisListType.X, OP.add)
        mw_b = mw.rearrange("p (t b) -> p b t", b=BL)
        xt6_b = xt.rearrange("p k (t b) -> p k b t", b=BL)
        for b in range(BL):
            nc.vector.tensor_scalar(xt6_b[0:1, 6, b, :], mw_b[:, b, :],
                                    mpred[0:1, b:b + 1], None, OP.subtract)

        # ---- projections + scans ----------------------------------------
        def projection(wih_dram, nk, rhs_of_k, xw):
            """xw[:, m, :] (bf16 [128, MT, SB]) = Wih_ext.T @ rhs (all t)."""
            for m in range(MT):
                wm = wpool.tile([128, nk, 128], BF16, tag="wihm")
                nc.sync.dma_start(out=wm[:, :, :], in_=wih_dram[m])
                for ch in range(2):
                    pp = pspool.tile([128, 512], F32, tag="proj")
                    for k in range(nk):
                        nc.tensor.matmul(pp[:, :], wm[:, k, :], rhs_of_k(k, ch),
                                         start=(k == 0), stop=(k == nk - 1))
                    nc.vector.tensor_copy(xw[:, m, 512 * ch:512 * (ch + 1)],
                                          pp[:, :])

        def scan_layer(whh_sb, xw, hdst):
            """Interleaved fwd/bwd 256-step scan. whh_sb/xw/hdst: dict d->tile"""
            hbuf, cbuf = {}, {}
            for d in ("f", "b"):
                hbuf[d] = rpool.tile([128, 2, KH, BL], BF16, tag=f"hbuf{d}", name=f"hbuf{d}")
                nc.vector.memset(hbuf[d][:, 0, :, :], 0.0)
                cbuf[d] = rpool.tile([128, KH, BL], F32, tag=f"cbuf{d}", name=f"cbuf{d}")
                nc.vector.memset(cbuf[d][:, :, :], 0.0)

            with tc.For_i(0, S, UNROLL, hint_engines=(mybir.EngineType.PE,)) as i:
                for j in range(UNROLL):
                    for d in ("f", "b"):
                        cur, nxt = j % 2, (j + 1) % 2
                        if d == "f":
                            col = i * BL + j * BL
                        else:
                            col = i * (-BL) + (S - 1 - j) * BL
                        # per-gate-group PSUM tiles, emitted g,i,f,o so the
                        # nonlinearity chain overlaps the remaining matmuls
                        gorder = (3, 0, 1, 2)          # g, i, f, o
                        pg = {}
                        for gg in gorder:
                            pg[gg] = psg.tile([128, KH, BL], F32, tag="gates",
                                              name=f"pg{gg}")
                            for mm in range(KH):
                                m = 4 * gg + mm
                                for k in range(KH):
                                    nc.tensor.matmul(
                                        pg[gg][:, mm, :],
                                        whh_sb[d][:, k, 128 * m:128 * (m + 1)],
                                        hbuf[d][:, cur, k, :],
                                        start=(k == 0), stop=(k == KH - 1))
                            if gg == 3:
                                gs3 = scpool.tile([128, KH, BL], F32, tag="gsb3")
                                nc.vector.tensor_tensor(
                                    gs3[:, :, :], pg[3][:, :, :],
                                    xw[d][:, 12:16, ds(col, BL)], OP.add)
                                tg = scpool.tile([128, KH, BL], F32, tag="tg")
                                nc.scalar.activation(tg[:, :, :], gs3[:, :, :],
                                                     AF.Tanh)
                            elif gg == 0:
                                gs0 = scpool.tile([128, KH, BL], F32, tag="gsb0")
                                nc.vector.tensor_tensor(
                                    gs0[:, :, :], pg[0][:, :, :],
                                    xw[d][:, 0:4, ds(col, BL)], OP.add)
                                si = scpool.tile([128, KH, BL], F32, tag="si")
                                nc.scalar.activation(si[:, :, :], gs0[:, :, :],
                                                     AF.Sigmoid)
                                t1 = scpool.tile([128, KH, BL], F32, tag="t1")
                                nc.vector.tensor_tensor(t1[:, :, :], si[:, :, :],
                                                        tg[:, :, :], OP.mult)
                            elif gg == 1:
                                gs1 = scpool.tile([128, KH, BL], F32, tag="gsb1")
                                nc.vector.tensor_tensor(
                                    gs1[:, :, :], pg[1][:, :, :],
                                    xw[d][:, 4:8, ds(col, BL)], OP.add)
                                sf = scpool.tile([128, KH, BL], F32, tag="sf")
                                nc.scalar.activation(sf[:, :, :], gs1[:, :, :],
                                                     AF.Sigmoid)
                                t2 = scpool.tile([128, KH, BL], F32, tag="t2")
                                nc.vector.tensor_tensor(t2[:, :, :], sf[:, :, :],
                                                        cbuf[d][:, :, :],
                                                        OP.mult)
                                nc.vector.tensor_tensor(cbuf[d][:, :, :],
                                                        t1[:, :, :], t2[:, :, :],
                                                        OP.add)
                                tcc = scpool.tile([128, KH, BL], F32, tag="tcc")
                                nc.scalar.activation(tcc[:, :, :],
                                                     cbuf[d][:, :, :], AF.Tanh)
                            else:
                                gs2 = scpool.tile([128, KH, BL], F32, tag="gsb2")
                                nc.vector.tensor_tensor(
                                    gs2[:, :, :], pg[2][:, :, :],
                                    xw[d][:, 8:12, ds(col, BL)], OP.add)
                                so = scpool.tile([128, KH, BL], F32, tag="so")
                                nc.scalar.activation(so[:, :, :], gs2[:, :, :],
                                                     AF.Sigmoid)
                                nc.vector.tensor_tensor(hbuf[d][:, nxt, :, :],
                                                        so[:, :, :],
                                                        tcc[:, :, :], OP.mult)
                        nc.vector.tensor_copy(hdst[d][:, :, ds(col, BL)],
                                              hbuf[d][:, nxt, :, :])

        # ---- layer 0 -----------------------------------------------------
        whh0 = {}
        for d in ("f", "b"):
            whh0[d] = wpool.tile([128, KH, G], BF16, tag="whh", name=f"whh0{d}")
            nc.sync.dma_start(out=whh0[d][:, :, :], in_=w[f"whh0{d}"][:, :, :])
        xw0 = {}
        for d in ("f", "b"):
            xw0[d] = xwpool.tile([128, MT, SB], BF16, tag="xw", name=f"xw0{d}")
            projection(w[f"wih0{d}"], K0,
                       lambda k, ch: xt[:, k, 512 * ch:512 * (ch + 1)],
                       xw0[d])
        h0 = {d: rpool.tile([128, KH, SB], BF16, tag=f"h0{d}", name=f"h0{d}")
              for d in ("f", "b")}
        scan_layer(whh0, xw0, h0)

        # ---- layer 1 -----------------------------------------------------
        whh1 = {}
        for d in ("f", "b"):
            whh1[d] = wpool.tile([128, KH, G], BF16, tag="whh", name=f"whh1{d}")
            nc.sync.dma_start(out=whh1[d][:, :, :], in_=w[f"whh1{d}"][:, :, :])

        def l1_rhs(k, ch):
            if k < KH:
                return h0["f"][:, k, 512 * ch:512 * (ch + 1)]
            if k < 2 * KH:
                return h0["b"][:, k - KH, 512 * ch:512 * (ch + 1)]
            return ones_row[:, 512 * ch:512 * (ch + 1)]

        xw1 = {}
        for d in ("f", "b"):
            xw1[d] = xwpool.tile([128, MT, SB], BF16, tag="xw", name=f"xw1{d}")
            projection(w[f"wih1{d}"], K1, l1_rhs, xw1[d])
        h1 = {d: rpool.tile([128, KH, SB], BF16, tag=f"h1{d}", name=f"h1{d}")
              for d in ("f", "b")}
        scan_layer(whh1, xw1, h1)

        # ---- output projection ------------------------------------------
        wo = wpool.tile([128, K1, C], BF16, tag="wout")
        nc.sync.dma_start(out=wo[:, :, :], in_=w["wout"][:, :, :])
        outT = rpool.tile([C, SB], F32, tag="outT")
        for ch in range(2):
            po = pspool.tile([C, 512], F32, tag="proj")
            for k in range(K1):
                if k < KH:
                    rhs = h1["f"][:, k, 512 * ch:512 * (ch + 1)]
                elif k < 2 * KH:
                    rhs = h1["b"][:, k - KH, 512 * ch:512 * (ch + 1)]
                else:
                    rhs = ones_row[:, 512 * ch:512 * (ch + 1)]
                nc.tensor.matmul(po[:, :], wo[:, k, :], rhs,
                                 start=(k == 0), stop=(k == K1 - 1))
            nc.vector.tensor_copy(outT[:, 512 * ch:512 * (ch + 1)], po[:, :])
        out_sbc = out.rearrange("b s c -> s b c")
        for cb in range(8):
            pt = pspool.tile([128, C], F32, tag="tp")
            nc.tensor.transpose(pt[:, :], outT[:, 128 * cb:128 * (cb + 1)],
                                ident[0:C, 0:C])
            onat = scpool.tile([128, C], F32, tag="onat")
            nc.vector.tensor_copy(onat[:, :], pt[:, :])
            nc.sync.dma_start(out=out_sbc[32 * cb:32 * (cb + 1), :, :],
                              in_=onat[:, :])


def _get_nc():
    if "nc" not in _cache:
        _cache["nc"] = build_nc()
    return _cache["nc"]


def kernel(**inputs):
    from concourse.bass_utils import run_bass_kernel_spmd

    wmaps = _prep_weights(inputs)
    hsf = np.asarray(inputs["hidden_states"], np.float32)
    rol = np.asarray(inputs["roles"])
    prd = np.asarray(inputs["predicates"])
    in_maps = []
    for c in range(NCORES):
        sl = slice(BL * c, BL * (c + 1))
        m = dict(wmaps)
        m["hs"] = _bf(hsf[:, sl])                                   # [4,BL,S,E]
        m["roles"] = np.ascontiguousarray(
            rol[sl].T.reshape(1, SB)).astype(np.float32)            # (t,b)
        m["preds"] = np.ascontiguousarray(
            prd[sl].T.reshape(1, SB)).astype(np.float32)
        in_maps.append(m)

    nc = _get_nc()
    res = run_bass_kernel_spmd(nc, in_maps, core_ids=list(range(NCORES)))
    return np.concatenate([r["out"] for r in res.results], axis=0)
